# revision 37
# baseline (speedup 1.0000x reference)
"""Causal multi-head attention on 8 Trainium2 NeuronCores.

Problem: B=4, T=2048, C=1024, H=16 heads (head_dim 64), causal softmax,
out = softmax(QK^T/8, causal) V projected by Wo, plus bias.

Sharding (hardcoded): 8 cores = 4 batches x 2 head-groups.  Core c handles
batch b = c//2 and heads g*8..g*8+7 where g = c%2 (tensor parallel over
heads: column-split Wq/Wk/Wv, row-split Wo).  Each core returns a partial
output [T, C] in bf16; the host sums the two head-group partials per batch
in f32 and adds the bias.

Device algorithm (per core), all in "transposed domain" so no on-chip
transposes are needed:
  xT [C, T] arrives head-dim-major (host passes x[b].T).
  qT = Wq_g^T x^T, kT = Wk_g^T x^T   [512, T]  (dims-on-partitions)
  v  = x Wv_g                        [T, 512]  (tokens-on-partitions)
  per head pair, per 512-wide q block, per 128-wide key tile:
    S^T = kT_h^T qT_h  (keys on partitions, two heads row-packed in the
          128x128 PE array via tile_position)
    E = exp(S^T / 8)  on ScalarE (PSUM -> SBUF bf16), causal-masked on the
        diagonal tiles with a triangular bf16 multiply
    ctx^T[h] (+= v_tile^T E) via PE.  The v stationary blocks are padded to
        128 columns (ones col | zeros | 64 v dims) so the weight loads stay
        FWL-eligible; the ones column at local col 0 makes PSUM row 0
        accumulate the softmax denominators.
  denominators are inverted straight out of the ctx PSUM row 0 by the
  custom-DVE fast approx reciprocal (partition base 0), replicated across
  partitions with gpsimd partition_broadcast (all on-chip; no DRAM round
  trip), and ctx^T is normalized and cast to bf16 on VectorE.
  partial = ctx^T^T Wo_g accumulated over the 4 head-pair K blocks.

Scheduling (the key to PE saturation; measured ~97% PE occupancy): the
kernel is PE-bound overall but ACT-paced inside the attention loop (exp of
[128,~1024] every iteration).  Engine queues are strict FIFO, so (a) any
matmul emitted while its input exp is in flight head-of-line blocks every
independent matmul behind it, and (b) emission order IS dependency order:
a consumer emitted before its producer reads garbage.  Structure:
  1. Windows (one window = one (pair, key-tile) iteration) are processed
     in PAIRS: both score matmul pairs back-to-back, then both (delayed)
     PV batches.  Each transition between the row-tiled score matmuls and
     full-array matmuls exposes one ~107ns LDWEIGHTS (row-group conflict
     blocks the PE's pull-ahead; FWL is off for 64-row stationaries), so
     halving the transitions saves ~110ns/window.
  2. The PV matmuls for window i are EMITTED in window i+1 — their exp
     finished a full window earlier, so they never stall the PE FIFO.
  3. Projection / output-projection matmuls are queued as fine-grained
     filler micro-ops, need-ordered, paced evenly over each block's
     windows, with FORCE-DRAINS of everything due before the window that
     consumes it (correctness independent of pacing).
  4. Attention starts as soon as pair 0's projections land (~10us, DMA
     need-ordered waves) instead of after all of block 0's (~43us).
  5. Normalize evacuates ctx PSUM banks with full-tile copies (a [128,N]
     DVE copy costs the same as [64,N]) so the next pair's PV never waits;
     the last pair uses a shortened variant plus held-back + direct-PSUM
     out-projection work to cover the reciprocal/broadcast chain, and the
     last block's j=3 chunks accumulate onto their partials inside the
     freed score banks (PE-side combine, no DVE add).
"""

import numpy as np

import concourse.bass as bass
import concourse.mybir as mybir
import concourse.tile as tile
from concourse import bacc
from concourse import library_config
from concourse.bass_utils import run_bass_kernel_spmd

F32 = mybir.dt.float32
BF16 = mybir.dt.bfloat16
AF = mybir.ActivationFunctionType

C = 1024
KP = C // 128  # k-tiles over the model dim


def build(S=2048, npair=4):
    """Emit the per-core program.  S = sequence length, npair = head pairs
    (the real problem uses S=2048, npair=4 -> 8 heads, 512 dims per core)."""
    CD = npair * 128        # q/k/v dims owned by this core
    HPC = npair * 2         # heads per core
    NJB = S // 512          # q blocks
    NMT = S // 128          # token tiles
    VW = 128                # per-head v block width (64 data + ones + pad)

    nc = bacc.Bacc("TRN2", target_bir_lowering=False, debug=False)
    xT = nc.dram_tensor("xT", [C, S], BF16, kind="ExternalInput").ap()
    wq = nc.dram_tensor("wq", [C, CD], BF16, kind="ExternalInput").ap()
    wk = nc.dram_tensor("wk", [C, CD], BF16, kind="ExternalInput").ap()
    wv = nc.dram_tensor("wv", [C, CD], BF16, kind="ExternalInput").ap()
    wo = nc.dram_tensor("wo", [CD, C], BF16, kind="ExternalInput").ap()
    out = nc.dram_tensor("out", [S, C], BF16, kind="ExternalOutput").ap()

    with tile.TileContext(nc) as tc:
        nc.gpsimd.load_library(library_config.attn)
        with tc.tile_pool(name="cpool", bufs=1) as cpool:
            # merged resident tiles so input DMAs batch into few descriptors
            xT_all = cpool.tile([128, KP * S], BF16, name="xTa", tag="xTa")
            wq_all = cpool.tile([128, KP * CD], BF16, name="wqa", tag="wqa")
            wk_all = cpool.tile([128, KP * CD], BF16, name="wka", tag="wka")
            wv_all = cpool.tile([128, KP * CD], BF16, name="wva", tag="wva")
            wo_all = cpool.tile([128, npair * C], BF16, name="woa", tag="woa")

            def xs(i, lo, hi):   # xT k-tile i, token cols lo:hi
                return xT_all[:, i * S + lo:i * S + hi]

            def wslice(w, i, lo, hi):   # w k-tile i, out-dim cols lo:hi
                return w[:, i * CD + lo:i * CD + hi]

            qT_bf = [[cpool.tile([128, 512], BF16, name=f"qTb{p}_{b}",
                                 tag=f"qTb{p}_{b}") for b in range(NJB)]
                     for p in range(npair)]
            kT_bf = [[cpool.tile([128, 512], BF16, name=f"kTb{p}_{b}",
                                 tag=f"kTb{p}_{b}") for b in range(NJB)]
                     for p in range(npair)]
            # v tiles: per head a 128-wide block [ones | 0 (63) | v_h (64)]:
            # the ones column at local col 0 makes PSUM row 0 of the ctx
            # matmul accumulate the softmax denominators right where the
            # custom-DVE reciprocal can read them (partition base 0), the
            # v data at cols 64..128 keeps the ctx rows legally aligned
            # (64 partitions at base 64), and the 128-wide stationary
            # keeps the PE fast-weight-load path on.
            v_bf = [cpool.tile([128, HPC * VW], BF16, name=f"vb{m}", tag=f"vb{m}")
                    for m in range(NMT)]
            ctxT_bf = [[cpool.tile([128, 512], BF16, name=f"cxb{p}_{b}",
                                   tag=f"cxb{p}_{b}") for b in range(NJB)]
                       for p in range(npair)]

            # ---- input DMAs: ~12 strided descriptors, need-ordered so the
            # first pair's working set (wq + x block 0 + wk + wv) lands
            # first; later x blocks and wo stream in behind compute ----
            def wsrc3(w):    # DRAM [C, CD] -> [128 p][KP i][CD c]
                return w.rearrange("(i p) c -> p i c", p=128)

            def wdst3(wt, cd):   # SBUF [128, KP*cd] -> [128][KP][cd]
                return wt.rearrange("p (i c) -> p i c", c=cd)

            xsrc = xT.rearrange("(i p) c -> p i c", p=128)
            xdst = xT_all.rearrange("p (i c) -> p i c", c=S)
            # First-wave descriptors are issued from DIFFERENT engine queues
            # so their ~0.6us issue costs run in parallel (the Sync queue
            # issues serially): pair-0 slices of wq/wk + x block 0 + wv land
            # ~4us sooner and attention starts correspondingly earlier.
            # Everything else streams behind the early fillers on Sync.
            nc.sync.dma_start(out=wdst3(wq_all, CD)[:, 0:4, 0:128],
                              in_=wsrc3(wq)[:, 0:4, 0:128])
            nc.sync.dma_start(out=xdst[:, 0:1, 0:512],
                              in_=xsrc[:, 0:1, 0:512])
            nc.sync.dma_start(out=wdst3(wq_all, CD)[:, 4:8, 0:128],
                              in_=wsrc3(wq)[:, 4:8, 0:128])
            for ks in (slice(1, 2), slice(2, 4), slice(4, 8)):
                nc.sync.dma_start(out=xdst[:, ks, 0:512],
                                  in_=xsrc[:, ks, 0:512])
            nc.sync.dma_start(out=wdst3(wk_all, CD)[:, :, 0:128],
                              in_=wsrc3(wk)[:, :, 0:128])
            for kh in range(2):
                k4 = slice(kh * 4, kh * 4 + 4)
                nc.sync.dma_start(out=wdst3(wv_all, CD)[:, k4, :],
                                  in_=wsrc3(wv)[:, k4, :])
            nc.sync.dma_start(out=wdst3(wq_all, CD)[:, :, 128:CD],
                              in_=wsrc3(wq)[:, :, 128:CD])
            nc.sync.dma_start(out=wdst3(wk_all, CD)[:, :, 128:CD],
                              in_=wsrc3(wk)[:, :, 128:CD])
            for b in range(1, NJB):
                nc.sync.dma_start(out=xdst[:, :, b * 512:(b + 1) * 512],
                                  in_=xsrc[:, :, b * 512:(b + 1) * 512])
            nc.sync.dma_start(
                out=wo_all.rearrange("p (j c) -> p j c", c=C),
                in_=wo.rearrange("(j p) c -> p j c", p=128))

            # lower-triangle (keep y>=p) bf16 mask for diagonal score tiles
            tri = cpool.tile([128, 128], BF16, name="tri", tag="tri")
            nc.gpsimd.memset(tri, 1.0)
            nc.gpsimd.affine_select(
                out=tri, in_=tri, pattern=[[1, 128]],
                compare_op=mybir.AluOpType.is_ge, fill=0.0, base=0,
                channel_multiplier=-1)
            # v tile init: zero the bookkeeping cols per head block and
            # set the ones column (data cols written by the projection cast)
            for m in range(NMT):
                vv = v_bf[m].rearrange("p (h x) -> p h x", x=VW)
                nc.gpsimd.memset(vv[:, :, 0:64], 0.0)
                nc.gpsimd.memset(vv[:, :, 0:1], 1.0)

            # ---- main pipeline ----
            with tc.tile_pool(name="psum", bufs=1, space="PSUM") as pp, \
                 tc.tile_pool(name="epool", bufs=6) as epool, \
                 tc.tile_pool(name="srp", bufs=2) as srp, \
                 tc.tile_pool(name="rbp", bufs=2) as rbp, \
                 tc.tile_pool(name="cfp", bufs=2) as cfp, \
                 tc.tile_pool(name="opp", bufs=8) as opp, \
                 tc.tile_pool(name="obuf", bufs=8) as obuf:

                # --- filler micro-ops: one closure ~= one engine instr ---
                def proj_qk_uops(jb, p, w_all, dstT):
                    st = {}

                    def mm(i):
                        def go():
                            if "ps" not in st:
                                st["ps"] = pp.tile([128, 512], F32, name="pj",
                                                   tag="pj", bufs=2)
                            nc.tensor.matmul(
                                st["ps"],
                                lhsT=wslice(w_all, i, p * 128, (p + 1) * 128),
                                rhs=xs(i, jb * 512, (jb + 1) * 512),
                                start=(i == 0), stop=(i == KP - 1))
                        return go

                    def cp():
                        nc.vector.tensor_copy(dstT[p][jb], st["ps"])
                    return [mm(i) for i in range(KP)] + [cp]

                def proj_v_uops(m):
                    st = {}

                    def mm(i):
                        def go():
                            if "ps" not in st:
                                st["ps"] = pp.tile([128, 512], F32, name="pv",
                                                   tag="pj", bufs=2)
                            nc.tensor.matmul(
                                st["ps"][:, 0:CD],
                                lhsT=xs(i, m * 128, (m + 1) * 128),
                                rhs=wslice(wv_all, i, 0, CD),
                                start=(i == 0), stop=(i == KP - 1))
                        return go

                    def cp():
                        vv = v_bf[m].rearrange("p (h x) -> p h x", x=VW)
                        src = st["ps"][:, 0:CD].rearrange("p (h x) -> p h x",
                                                          x=64)
                        nc.vector.tensor_copy(vv[:, :, 64:VW], src)
                    return [mm(i) for i in range(KP)] + [cp]

                def outproj_uops(mt, nh):
                    st = {}

                    def mm(j):
                        def go():
                            if "ps" not in st:
                                st["ps"] = pp.tile([128, 512], F32, name="po",
                                                   tag="pj", bufs=2)
                            nc.tensor.matmul(
                                st["ps"],
                                lhsT=ctxT_bf[j][mt // 4][
                                    :, (mt % 4) * 128:(mt % 4 + 1) * 128],
                                rhs=wo_all[:, j * C + nh * 512:
                                           j * C + (nh + 1) * 512],
                                start=(j == 0), stop=(j == npair - 1))
                        return go

                    def cpdma(cpeng="vector"):
                        ot = obuf.tile([128, 512], BF16, name="ot", tag="ot")
                        if cpeng == "scalar":
                            nc.scalar.activation(ot, st["ps"], AF.Copy,
                                                 scale=1.0)
                        else:
                            nc.vector.tensor_copy(ot, st["ps"])
                        nc.sync.dma_start(
                            out=out[mt * 128:(mt + 1) * 128,
                                    nh * 512:(nh + 1) * 512],
                            in_=ot)
                    return [mm(j) for j in range(4)] + [cpdma]

                def outproj3_part_uops(mt, nh, st):
                    # two-level accumulation for the last block: pairs 0..2
                    # into PSUM then parked in SBUF, so only the j=3 chunk
                    # remains for the tail and no PSUM slot is held across
                    # it.  These run as ordinary jb3 fillers once pair 2's
                    # ctx^T is normalized (~3/4 through the block).
                    def mm(j):
                        def go():
                            if "ps" not in st:
                                st["ps"] = pp.tile([128, 512], F32, name="p3",
                                                   tag="pj", bufs=2)
                            nc.tensor.matmul(
                                st["ps"],
                                lhsT=ctxT_bf[j][mt // 4][
                                    :, (mt % 4) * 128:(mt % 4 + 1) * 128],
                                rhs=wo_all[:, j * C + nh * 512:
                                           j * C + (nh + 1) * 512],
                                start=(j == 0), stop=(j == 2))
                        return go

                    def park():
                        st["po"] = opp.tile([128, 512], F32, name="po3",
                                            tag="po3")
                        nc.vector.tensor_copy(st["po"], st["ps"])
                        del st["ps"]
                    return [mm(j) for j in range(3)] + [park]

                def outproj3_setup_tail(mt, nh, st):
                    def mm3():
                        st["ps"] = pp.tile([128, 512], F32, name="p3t",
                                           tag="pj", bufs=2)
                        nc.tensor.matmul(
                            st["ps"],
                            lhsT=ctxT_bf[3][mt // 4][
                                :, (mt % 4) * 128:(mt % 4 + 1) * 128],
                            rhs=wo_all[:, 3 * C + nh * 512:
                                       3 * C + (nh + 1) * 512],
                            start=True, stop=True)

                    def adddma():
                        ot = obuf.tile([128, 512], BF16, name="ot", tag="ot")
                        nc.vector.tensor_add(ot, st["ps"], st["po"])
                        eng = nc.sync if nh == 0 else nc.gpsimd
                        eng.dma_start(
                            out=out[mt * 128:(mt + 1) * 128,
                                    nh * 512:(nh + 1) * 512],
                            in_=ot)
                    st["mm3"] = mm3
                    st["adddma"] = adddma

                # ---- window stream ----
                # head: only pair 0's q/k + block-0 v run before attention
                # starts; everything else drains as deadline-ordered fillers.
                for u in proj_qk_uops(0, 0, wq_all, qT_bf):
                    u()
                for u in proj_qk_uops(0, 0, wk_all, kT_bf):
                    u()
                for m in range(4):
                    for u in proj_v_uops(m):
                        u()

                # filler queue per q-block phase, need-ordered:
                #   fill[0]: q/k for pairs 1-3 of block 0 (deadlines: their
                #            own jb0 windows), then pair 0 of block 1, then
                #            v(block 1) (needed by jb1's diagonal windows)
                #   fill[jb]: q/k pairs 1-3 of this block, pair 0 of next,
                #            v(next block)
                #   fill[3]: q/k pairs 1-3 of block 3, then all out-
                #            projections for blocks 0-2 (block 3 goes
                #            through the endgame path)
                def qk(jb, p):
                    return (proj_qk_uops(jb, p, wq_all, qT_bf)
                            + proj_qk_uops(jb, p, wk_all, kT_bf))

                op3_states = [{} for _ in range(8)]
                op3_groups = [(mt, nh) for mt in range(12, 16)
                              for nh in range(2)]

                # queue items are (uop, due, ready): `due` = (p, kt2) of
                # the first window whose scores/PV consume the uop's output
                # -- the loop FORCE-DRAINS everything due before emitting
                # that window, making correctness independent of pacing
                # (emission order IS execution order per engine; a consumer
                # emitted before its producer reads garbage).  `ready` =
                # earliest window the uop may be emitted (used by the op3
                # partials, which read pair 2's normalized ctx^T).
                fill = {jb: [] for jb in range(NJB)}
                for jb in range(NJB):
                    for p in range(1, npair):
                        fill[jb] += [(u, (p, 0), None) for u in qk(jb, p)]
                    if jb + 1 < NJB:
                        fill[jb] += [(u, None, None) for u in qk(jb + 1, 0)]
                        for m in range(4 * (jb + 1), 4 * (jb + 1) + 4):
                            fill[jb] += [(u, None, None)
                                         for u in proj_v_uops(m)]
                # 4 outproj groups of block 2 are held back to give the PE
                # work during the final pair's normalize chain; the op3
                # park-partials go last so they never emit before pair 2 of
                # block 3 is normalized (~3/4 through jb3).  op3 groups 0-3
                # take the direct-PSUM endgame path instead (their j=3 chunk
                # accumulates onto the partial inside PSUM, killing the DVE
                # add) using the score banks, which are free by then.
                held_back = []
                for b in range(NJB - 1):
                    for mt in range(4 * b, 4 * b + 4):
                        for nh in range(2):
                            g = outproj_uops(mt, nh)
                            if b == NJB - 2 and mt >= 4 * b + 2:
                                held_back.append(g)
                            else:
                                fill[NJB - 1] += [(u, None, None) for u in g]
                NDIRECT = 8

                # pending PV work from the previous window: list of closures
                pend_pv = []
                pend_norm = []

                def emit_scores(jb, p, kt, c0, c1, nkt):
                    qoff = max(0, kt * 128 - jb * 512)
                    N = 512 - qoff
                    ps2 = pp.tile([128, 1024], F32, name="ps2", tag="pa",
                                  bufs=2)
                    kts = kT_bf[p][kt // 4][:, (kt % 4) * 128:
                                                (kt % 4 + 1) * 128]
                    qts = qT_bf[p][jb][:, qoff:qoff + N]
                    nc.tensor.matmul(
                        ps2[:, 0:N],
                        lhsT=kts[0:64, :], rhs=qts[0:64, :],
                        start=True, stop=True)
                    nc.tensor.matmul(
                        ps2[:, 512:512 + N],
                        lhsT=kts[64:128, :], rhs=qts[64:128, :],
                        start=True, stop=True, tile_position=(64, 0))
                    et = epool.tile([128, 1024], BF16, name="et", tag="et")
                    if qoff == 0:
                        nc.scalar.activation(et, ps2, AF.Exp, scale=0.125)
                    else:
                        e3 = et.rearrange("p (h x) -> p h x", x=512)
                        p3 = ps2.rearrange("p (h x) -> p h x", x=512)
                        nc.scalar.activation(
                            e3[:, :, 0:N], p3[:, :, 0:N],
                            AF.Exp, scale=0.125)
                    if kt * 128 >= jb * 512:
                        for hh in range(2):
                            nc.vector.tensor_mul(
                                et[:, hh * 512:hh * 512 + 128],
                                et[:, hh * 512:hh * 512 + 128],
                                tri)

                    def pv():
                        h0, h1 = 2 * p, 2 * p + 1
                        nc.tensor.matmul(
                            c0[:, qoff:512],
                            lhsT=v_bf[kt][:, h0 * VW:(h0 + 1) * VW],
                            rhs=et[:, 0:N],
                            start=(kt == 0), stop=(kt == nkt - 1))
                        nc.tensor.matmul(
                            c1[:, qoff:512],
                            lhsT=v_bf[kt][:, h1 * VW:(h1 + 1) * VW],
                            rhs=et[:, 512:512 + N],
                            start=(kt == 0), stop=(kt == nkt - 1))
                    return pv

                def emit_normalize_last(jb, p, c0, c1):
                    # endgame variant: nothing needs the ctx PSUM banks
                    # afterwards, so skip the evac-first ordering and put
                    # the reciprocals (read PSUM row 0 directly) at the
                    # head of the DVE queue — the partition broadcast then
                    # starts ~1.2us earlier and the whole chain shortens
                    cx0 = cfp.tile([128, 512], F32, name="cx0", tag="cx")
                    cx1 = cfp.tile([128, 512], F32, name="cx1", tag="cx")
                    rr2 = srp.tile([1, 1024], F32, name="rr2", tag="rr")
                    r64w = rbp.tile([128, 1024], F32, name="r64w", tag="rw")
                    nc.vector.reciprocal_approx_fast(
                        out=rr2[0:1, 0:512], in_=c0[0:1, :])
                    nc.vector.reciprocal_approx_fast(
                        out=rr2[0:1, 512:1024], in_=c1[0:1, :])
                    nc.vector.tensor_copy(cx0, c0)
                    nc.vector.tensor_copy(cx1, c1)
                    nc.gpsimd.partition_broadcast(r64w, rr2, channels=128)
                    nc.vector.tensor_mul(
                        ctxT_bf[p][jb][0:64, :], cx0[64:128, :],
                        r64w[64:128, 0:512])
                    nc.vector.tensor_mul(
                        ctxT_bf[p][jb][64:128, :], cx1[64:128, :],
                        r64w[64:128, 512:1024])

                def emit_normalize(jb, p, c0, c1):
                    # full-tile evacuations first: a [128,512] DVE copy costs
                    # the same as a [64,512] one (partition-parallel), and it
                    # frees each ctx PSUM bank in one op (~0.7us) instead of
                    # holding it through the reciprocal chain, so the next
                    # pair's first PV never waits.  The reciprocal then reads
                    # the denominator row from SBUF.
                    cx0 = cfp.tile([128, 512], F32, name="cx0", tag="cx")
                    cx1 = cfp.tile([128, 512], F32, name="cx1", tag="cx")
                    rr2 = srp.tile([1, 1024], F32, name="rr2", tag="rr")
                    r64w = rbp.tile([128, 1024], F32, name="r64w", tag="rw")
                    nc.vector.tensor_copy(cx0, c0)
                    nc.vector.tensor_copy(cx1, c1)
                    nc.vector.reciprocal_approx_fast(
                        out=rr2[0:1, 0:512], in_=cx0[0:1, :])
                    nc.vector.reciprocal_approx_fast(
                        out=rr2[0:1, 512:1024], in_=cx1[0:1, :])
                    nc.gpsimd.partition_broadcast(r64w, rr2, channels=128)
                    nc.vector.tensor_mul(
                        ctxT_bf[p][jb][0:64, :], cx0[64:128, :],
                        r64w[64:128, 0:512])
                    nc.vector.tensor_mul(
                        ctxT_bf[p][jb][64:128, :], cx1[64:128, :],
                        r64w[64:128, 512:1024])

                # global loop over window PAIRS: both score-pairs of two
                # consecutive key tiles are emitted back-to-back, then both
                # delayed PV batches.  Each transition between the row-tiled
                # score matmuls and full-array matmuls exposes one ~107ns
                # LDWEIGHTS (row-group conflict blocks the pull-ahead), so
                # halving the number of transitions saves ~110ns per window.
                for jb in range(NJB):
                    nkt = 4 * (jb + 1)
                    nwp = npair * nkt // 2
                    fillers = fill[jb]
                    it = 0
                    for p in range(npair):
                        c0 = pp.tile([128, 512], F32, name="c0", tag="pc",
                                     bufs=2)
                        c1 = pp.tile([128, 512], F32, name="c1", tag="pc",
                                     bufs=2)
                        for kt2 in range(0, nkt, 2):
                            # force-drain every filler due at this window
                            while (fillers and fillers[0][1] is not None
                                   and fillers[0][1] <= (p, kt2)):
                                fillers.pop(0)[0]()
                            pv_a = emit_scores(jb, p, kt2, c0, c1, nkt)
                            pv_b = emit_scores(jb, p, kt2 + 1, c0, c1, nkt)
                            # delayed PVs from the previous window pair
                            # (their exps finished a window ago -> no stall)
                            for f in pend_pv:
                                f()
                            pend_pv.clear()
                            for args in pend_norm:
                                emit_normalize(*args)
                            pend_norm.clear()
                            pend_pv += [pv_a, pv_b]
                            if kt2 + 1 == nkt - 1:
                                pend_norm.append((jb, p, c0, c1))
                            # evenly paced fillers
                            it += 1
                            if fillers:
                                nf = -(-len(fillers) // max(1, nwp - it + 1))
                                for _ in range(min(nf, 8, len(fillers))):
                                    if (fillers[0][2] is not None
                                            and fillers[0][2] > (p, kt2)):
                                        break
                                    fillers.pop(0)[0]()

                    # drain any leftover fillers before the next block
                    for u, _, _ in fillers:
                        u()
                    fill[jb] = []

                # endgame: flush the last PV batch, then the held-back
                # outproj matmuls (PE work that covers the final normalize
                # chain; their DVE copies are deferred so they don't delay
                # it), the fast last-pair normalize, the deferred copies,
                # and finally the j=3 tails + adds + stores
                for f in pend_pv:
                    f()
                pend_pv.clear()
                # held-back groups: copies pipeline one group behind the
                # matmuls (on ScalarE) so their PSUM slots recycle without
                # gating the later direct-path work
                for i, grp in enumerate(held_back):
                    for u in grp[:-1]:
                        u()
                    if i >= 1:
                        held_back[i - 1][-1]("scalar")
                held_back[-1][-1]("scalar")

                def wo_sl(j, nh):
                    return wo_all[:, j * C + nh * 512:j * C + (nh + 1) * 512]

                def cx_sl(j, mt):
                    return ctxT_bf[j][mt // 4][:, (mt % 4) * 128:
                                               (mt % 4 + 1) * 128]

                # direct-path partials (j=0..2) into the freed score banks;
                # together with the held-back matmuls they keep the PE busy
                # through the final normalize chain
                direct_ps = []
                pa_t = None
                for g in range(NDIRECT):
                    mt, nh = op3_groups[g]
                    if g < 4:
                        if g % 2 == 0:
                            pa_t = pp.tile([128, 1024], F32, name="p3d",
                                           tag="pa", bufs=2)
                        ps = pa_t[:, (g % 2) * 512:(g % 2 + 1) * 512]
                    elif g < 6:
                        ps = pp.tile([128, 512], F32, name="p3c", tag="pc",
                                     bufs=2)
                    else:
                        ps = pp.tile([128, 512], F32, name="p3j", tag="pj",
                                     bufs=2)
                    direct_ps.append(ps)
                    for j in range(3):
                        nc.tensor.matmul(ps, lhsT=cx_sl(j, mt),
                                         rhs=wo_sl(j, nh), start=(j == 0),
                                         stop=(j == 2), skip_group_check=True)
                # pend_norm holds the last pair; use the fast variant
                emit_normalize_last(*pend_norm.pop())
                assert not pend_norm
                # direct tails first: accumulate j=3 onto the partial inside
                # PSUM, then a single copy + store.  Copies alternate
                # ScalarE/VectorE and store issues alternate the Sync/GpSimd
                # queues so neither serial path gates the drain.
                for g in range(NDIRECT):
                    mt, nh = op3_groups[g]
                    nc.tensor.matmul(direct_ps[g], lhsT=cx_sl(3, mt),
                                     rhs=wo_sl(3, nh), start=False,
                                     stop=True, skip_group_check=True)
                for g in range(NDIRECT):
                    mt, nh = op3_groups[g]
                    ot = obuf.tile([128, 512], BF16, name="ot", tag="ot")
                    if g % 2 == 0:
                        nc.scalar.activation(ot, direct_ps[g], AF.Copy,
                                             scale=1.0)
                    else:
                        nc.vector.tensor_copy(ot, direct_ps[g])
                    eng = nc.gpsimd if g % 2 == 0 else nc.sync
                    eng.dma_start(
                        out=out[mt * 128:(mt + 1) * 128,
                                nh * 512:(nh + 1) * 512],
                        in_=ot)

    nc.compile()
    return nc


_NC_CACHE = {}


def _get_nc(S=2048, npair=4):
    key = (S, npair)
    if key not in _NC_CACHE:
        _NC_CACHE[key] = build(S, npair)
    return _NC_CACHE[key]


def make_in_maps(x, Wq, Wk, Wv, Wo):
    """Host-side sharding: batch x head-group slices, x transposed to
    dims-major layout, rounded to bf16 (the dtype the device matmuls use)."""
    import ml_dtypes

    bf = ml_dtypes.bfloat16
    in_maps = []
    for c in range(8):
        b, g = divmod(c, 2)
        sl = slice(g * 512, (g + 1) * 512)
        in_maps.append({
            "xT": np.ascontiguousarray(x[b].T).astype(bf),
            "wq": np.ascontiguousarray(Wq[:, sl]).astype(bf),
            "wk": np.ascontiguousarray(Wk[:, sl]).astype(bf),
            "wv": np.ascontiguousarray(Wv[:, sl]).astype(bf),
            "wo": np.ascontiguousarray(Wo[sl, :]).astype(bf),
        })
    return in_maps


def run_cores(x, Wq, Wk, Wv, Wo, trace=False, trace_kwargs=None):
    nc = _get_nc(2048, 4)
    in_maps = make_in_maps(x, Wq, Wk, Wv, Wo)
    return run_bass_kernel_spmd(
        nc, in_maps, core_ids=list(range(8)), trace=trace,
        trace_kwargs=trace_kwargs or {})


def kernel(x, Wq, Wk, Wv, Wo, bo):
    x = np.asarray(x, dtype=np.float32)
    Wq = np.asarray(Wq, dtype=np.float32)
    Wk = np.asarray(Wk, dtype=np.float32)
    Wv = np.asarray(Wv, dtype=np.float32)
    Wo = np.asarray(Wo, dtype=np.float32)
    bo = np.asarray(bo, dtype=np.float32)

    res = run_cores(x, Wq, Wk, Wv, Wo).results
    out = np.empty((4, 2048, 1024), dtype=np.float32)
    for b in range(4):
        out[b] = (res[2 * b]["out"].astype(np.float32)
                  + res[2 * b + 1]["out"].astype(np.float32) + bo[None, :])
    return out


# revision 38
# speedup vs baseline: 1.0005x; 1.0005x over previous
"""Causal multi-head attention on 8 Trainium2 NeuronCores.

Problem: B=4, T=2048, C=1024, H=16 heads (head_dim 64), causal softmax,
out = softmax(QK^T/8, causal) V projected by Wo, plus bias.

Sharding (hardcoded): 8 cores = 4 batches x 2 head-groups.  Core c handles
batch b = c//2 and heads g*8..g*8+7 where g = c%2 (tensor parallel over
heads: column-split Wq/Wk/Wv, row-split Wo).  Each core returns a partial
output [T, C] in bf16; the host sums the two head-group partials per batch
in f32 and adds the bias.

Device algorithm (per core), all in "transposed domain" so no on-chip
transposes are needed:
  xT [C, T] arrives head-dim-major (host passes x[b].T).
  qT = Wq_g^T x^T, kT = Wk_g^T x^T   [512, T]  (dims-on-partitions)
  v  = x Wv_g                        [T, 512]  (tokens-on-partitions)
  per head pair, per 512-wide q block, per 128-wide key tile:
    S^T = kT_h^T qT_h  (keys on partitions, two heads row-packed in the
          128x128 PE array via tile_position)
    E = exp(S^T / 8)  on ScalarE (PSUM -> SBUF bf16), causal-masked on the
        diagonal tiles with a triangular bf16 multiply
    ctx^T[h] (+= v_tile^T E) via PE.  The v stationary blocks are padded to
        128 columns (ones col | zeros | 64 v dims) so the weight loads stay
        FWL-eligible; the ones column at local col 0 makes PSUM row 0
        accumulate the softmax denominators.
  denominators are inverted straight out of the ctx PSUM row 0 by the
  custom-DVE fast approx reciprocal (partition base 0), replicated across
  partitions with gpsimd partition_broadcast (all on-chip; no DRAM round
  trip), and ctx^T is normalized and cast to bf16 on VectorE.
  partial = ctx^T^T Wo_g accumulated over the 4 head-pair K blocks.

Scheduling (the key to PE saturation; measured ~97% PE occupancy): the
kernel is PE-bound overall but ACT-paced inside the attention loop (exp of
[128,~1024] every iteration).  Engine queues are strict FIFO, so (a) any
matmul emitted while its input exp is in flight head-of-line blocks every
independent matmul behind it, and (b) emission order IS dependency order:
a consumer emitted before its producer reads garbage.  Structure:
  1. Windows (one window = one (pair, key-tile) iteration) are processed
     in PAIRS: both score matmul pairs back-to-back, then both (delayed)
     PV batches.  Each transition between the row-tiled score matmuls and
     full-array matmuls exposes one ~107ns LDWEIGHTS (row-group conflict
     blocks the PE's pull-ahead; FWL is off for 64-row stationaries), so
     halving the transitions saves ~110ns/window.
  2. The PV matmuls for window i are EMITTED in window i+1 — their exp
     finished a full window earlier, so they never stall the PE FIFO.
  3. Projection / output-projection matmuls are queued as fine-grained
     filler micro-ops, need-ordered, paced evenly over each block's
     windows, with FORCE-DRAINS of everything due before the window that
     consumes it (correctness independent of pacing).
  4. Attention starts as soon as pair 0's projections land (~10us, DMA
     need-ordered waves) instead of after all of block 0's (~43us).
  5. Normalize evacuates ctx PSUM banks with full-tile copies (a [128,N]
     DVE copy costs the same as [64,N]) so the next pair's PV never waits;
     the last pair uses a shortened variant plus held-back + direct-PSUM
     out-projection work to cover the reciprocal/broadcast chain, and the
     last block's j=3 chunks accumulate onto their partials inside the
     freed score banks (PE-side combine, no DVE add).
"""

import numpy as np

import concourse.bass as bass
import concourse.mybir as mybir
import concourse.tile as tile
from concourse import bacc
from concourse import library_config
from concourse.bass_utils import run_bass_kernel_spmd

F32 = mybir.dt.float32
BF16 = mybir.dt.bfloat16
AF = mybir.ActivationFunctionType

C = 1024
KP = C // 128  # k-tiles over the model dim


def build(S=2048, npair=4):
    """Emit the per-core program.  S = sequence length, npair = head pairs
    (the real problem uses S=2048, npair=4 -> 8 heads, 512 dims per core)."""
    CD = npair * 128        # q/k/v dims owned by this core
    HPC = npair * 2         # heads per core
    NJB = S // 512          # q blocks
    NMT = S // 128          # token tiles
    VW = 128                # per-head v block width (64 data + ones + pad)

    nc = bacc.Bacc("TRN2", target_bir_lowering=False, debug=False)
    xT = nc.dram_tensor("xT", [C, S], BF16, kind="ExternalInput").ap()
    wq = nc.dram_tensor("wq", [C, CD], BF16, kind="ExternalInput").ap()
    wk = nc.dram_tensor("wk", [C, CD], BF16, kind="ExternalInput").ap()
    wv = nc.dram_tensor("wv", [C, CD], BF16, kind="ExternalInput").ap()
    wo = nc.dram_tensor("wo", [CD, C], BF16, kind="ExternalInput").ap()
    out = nc.dram_tensor("out", [S, C], BF16, kind="ExternalOutput").ap()

    with tile.TileContext(nc) as tc:
        nc.gpsimd.load_library(library_config.attn)
        with tc.tile_pool(name="cpool", bufs=1) as cpool:
            # merged resident tiles so input DMAs batch into few descriptors
            xT_all = cpool.tile([128, KP * S], BF16, name="xTa", tag="xTa")
            wq_all = cpool.tile([128, KP * CD], BF16, name="wqa", tag="wqa")
            wk_all = cpool.tile([128, KP * CD], BF16, name="wka", tag="wka")
            wv_all = cpool.tile([128, KP * CD], BF16, name="wva", tag="wva")
            wo_all = cpool.tile([128, npair * C], BF16, name="woa", tag="woa")

            def xs(i, lo, hi):   # xT k-tile i, token cols lo:hi
                return xT_all[:, i * S + lo:i * S + hi]

            def wslice(w, i, lo, hi):   # w k-tile i, out-dim cols lo:hi
                return w[:, i * CD + lo:i * CD + hi]

            qT_bf = [[cpool.tile([128, 512], BF16, name=f"qTb{p}_{b}",
                                 tag=f"qTb{p}_{b}") for b in range(NJB)]
                     for p in range(npair)]
            kT_bf = [[cpool.tile([128, 512], BF16, name=f"kTb{p}_{b}",
                                 tag=f"kTb{p}_{b}") for b in range(NJB)]
                     for p in range(npair)]
            # v tiles: per head a 128-wide block [ones | 0 (63) | v_h (64)]:
            # the ones column at local col 0 makes PSUM row 0 of the ctx
            # matmul accumulate the softmax denominators right where the
            # custom-DVE reciprocal can read them (partition base 0), the
            # v data at cols 64..128 keeps the ctx rows legally aligned
            # (64 partitions at base 64), and the 128-wide stationary
            # keeps the PE fast-weight-load path on.
            v_bf = [cpool.tile([128, HPC * VW], BF16, name=f"vb{m}", tag=f"vb{m}")
                    for m in range(NMT)]
            ctxT_bf = [[cpool.tile([128, 512], BF16, name=f"cxb{p}_{b}",
                                   tag=f"cxb{p}_{b}") for b in range(NJB)]
                       for p in range(npair)]

            # ---- input DMAs: ~12 strided descriptors, need-ordered so the
            # first pair's working set (wq + x block 0 + wk + wv) lands
            # first; later x blocks and wo stream in behind compute ----
            def wsrc3(w):    # DRAM [C, CD] -> [128 p][KP i][CD c]
                return w.rearrange("(i p) c -> p i c", p=128)

            def wdst3(wt, cd):   # SBUF [128, KP*cd] -> [128][KP][cd]
                return wt.rearrange("p (i c) -> p i c", c=cd)

            xsrc = xT.rearrange("(i p) c -> p i c", p=128)
            xdst = xT_all.rearrange("p (i c) -> p i c", c=S)
            # First-wave descriptors are issued from DIFFERENT engine queues
            # so their ~0.6us issue costs run in parallel (the Sync queue
            # issues serially): pair-0 slices of wq/wk + x block 0 + wv land
            # ~4us sooner and attention starts correspondingly earlier.
            # Everything else streams behind the early fillers on Sync.
            nc.sync.dma_start(out=wdst3(wq_all, CD)[:, 0:4, 0:128],
                              in_=wsrc3(wq)[:, 0:4, 0:128])
            nc.sync.dma_start(out=xdst[:, 0:1, 0:512],
                              in_=xsrc[:, 0:1, 0:512])
            nc.sync.dma_start(out=wdst3(wq_all, CD)[:, 4:8, 0:128],
                              in_=wsrc3(wq)[:, 4:8, 0:128])
            for ks in (slice(1, 2), slice(2, 4), slice(4, 8)):
                nc.sync.dma_start(out=xdst[:, ks, 0:512],
                                  in_=xsrc[:, ks, 0:512])
            nc.sync.dma_start(out=wdst3(wk_all, CD)[:, :, 0:128],
                              in_=wsrc3(wk)[:, :, 0:128])
            for kh in range(2):
                k4 = slice(kh * 4, kh * 4 + 4)
                nc.sync.dma_start(out=wdst3(wv_all, CD)[:, k4, :],
                                  in_=wsrc3(wv)[:, k4, :])
            nc.sync.dma_start(out=wdst3(wq_all, CD)[:, :, 128:CD],
                              in_=wsrc3(wq)[:, :, 128:CD])
            nc.sync.dma_start(out=wdst3(wk_all, CD)[:, :, 128:CD],
                              in_=wsrc3(wk)[:, :, 128:CD])
            for b in range(1, NJB):
                nc.sync.dma_start(out=xdst[:, :, b * 512:(b + 1) * 512],
                                  in_=xsrc[:, :, b * 512:(b + 1) * 512])
            nc.sync.dma_start(
                out=wo_all.rearrange("p (j c) -> p j c", c=C),
                in_=wo.rearrange("(j p) c -> p j c", p=128))

            # lower-triangle (keep y>=p) bf16 mask for diagonal score tiles
            tri = cpool.tile([128, 128], BF16, name="tri", tag="tri")
            nc.gpsimd.memset(tri, 1.0)
            nc.gpsimd.affine_select(
                out=tri, in_=tri, pattern=[[1, 128]],
                compare_op=mybir.AluOpType.is_ge, fill=0.0, base=0,
                channel_multiplier=-1)
            # v tile init: zero the bookkeeping cols per head block and
            # set the ones column (data cols written by the projection cast)
            for m in range(NMT):
                vv = v_bf[m].rearrange("p (h x) -> p h x", x=VW)
                nc.gpsimd.memset(vv[:, :, 0:64], 0.0)
                nc.gpsimd.memset(vv[:, :, 0:1], 1.0)

            # ---- main pipeline ----
            with tc.tile_pool(name="psum", bufs=1, space="PSUM") as pp, \
                 tc.tile_pool(name="epool", bufs=6) as epool, \
                 tc.tile_pool(name="srp", bufs=2) as srp, \
                 tc.tile_pool(name="rbp", bufs=2) as rbp, \
                 tc.tile_pool(name="cfp", bufs=2) as cfp, \
                 tc.tile_pool(name="opp", bufs=8) as opp, \
                 tc.tile_pool(name="obuf", bufs=8) as obuf:

                # --- filler micro-ops: one closure ~= one engine instr ---
                def proj_qk_uops(jb, p, w_all, dstT):
                    st = {}

                    def mm(i):
                        def go():
                            if "ps" not in st:
                                st["ps"] = pp.tile([128, 512], F32, name="pj",
                                                   tag="pj", bufs=2)
                            nc.tensor.matmul(
                                st["ps"],
                                lhsT=wslice(w_all, i, p * 128, (p + 1) * 128),
                                rhs=xs(i, jb * 512, (jb + 1) * 512),
                                start=(i == 0), stop=(i == KP - 1))
                        return go

                    def cp():
                        nc.vector.tensor_copy(dstT[p][jb], st["ps"])
                    return [mm(i) for i in range(KP)] + [cp]

                def proj_v_uops(m):
                    st = {}

                    def mm(i):
                        def go():
                            if "ps" not in st:
                                st["ps"] = pp.tile([128, 512], F32, name="pv",
                                                   tag="pj", bufs=2)
                            nc.tensor.matmul(
                                st["ps"][:, 0:CD],
                                lhsT=xs(i, m * 128, (m + 1) * 128),
                                rhs=wslice(wv_all, i, 0, CD),
                                start=(i == 0), stop=(i == KP - 1))
                        return go

                    def cp():
                        vv = v_bf[m].rearrange("p (h x) -> p h x", x=VW)
                        src = st["ps"][:, 0:CD].rearrange("p (h x) -> p h x",
                                                          x=64)
                        nc.vector.tensor_copy(vv[:, :, 64:VW], src)
                    return [mm(i) for i in range(KP)] + [cp]

                def outproj_uops(mt, nh):
                    st = {}

                    def mm(j):
                        def go():
                            if "ps" not in st:
                                st["ps"] = pp.tile([128, 512], F32, name="po",
                                                   tag="pj", bufs=2)
                            nc.tensor.matmul(
                                st["ps"],
                                lhsT=ctxT_bf[j][mt // 4][
                                    :, (mt % 4) * 128:(mt % 4 + 1) * 128],
                                rhs=wo_all[:, j * C + nh * 512:
                                           j * C + (nh + 1) * 512],
                                start=(j == 0), stop=(j == npair - 1))
                        return go

                    def cpdma(cpeng="vector"):
                        ot = obuf.tile([128, 512], BF16, name="ot", tag="ot")
                        if cpeng == "scalar":
                            nc.scalar.activation(ot, st["ps"], AF.Copy,
                                                 scale=1.0)
                        else:
                            nc.vector.tensor_copy(ot, st["ps"])
                        nc.sync.dma_start(
                            out=out[mt * 128:(mt + 1) * 128,
                                    nh * 512:(nh + 1) * 512],
                            in_=ot)
                    return [mm(j) for j in range(4)] + [cpdma]

                def outproj3_part_uops(mt, nh, st):
                    # two-level accumulation for the last block: pairs 0..2
                    # into PSUM then parked in SBUF, so only the j=3 chunk
                    # remains for the tail and no PSUM slot is held across
                    # it.  These run as ordinary jb3 fillers once pair 2's
                    # ctx^T is normalized (~3/4 through the block).
                    def mm(j):
                        def go():
                            if "ps" not in st:
                                st["ps"] = pp.tile([128, 512], F32, name="p3",
                                                   tag="pj", bufs=2)
                            nc.tensor.matmul(
                                st["ps"],
                                lhsT=ctxT_bf[j][mt // 4][
                                    :, (mt % 4) * 128:(mt % 4 + 1) * 128],
                                rhs=wo_all[:, j * C + nh * 512:
                                           j * C + (nh + 1) * 512],
                                start=(j == 0), stop=(j == 2))
                        return go

                    def park():
                        st["po"] = opp.tile([128, 512], F32, name="po3",
                                            tag="po3")
                        nc.vector.tensor_copy(st["po"], st["ps"])
                        del st["ps"]
                    return [mm(j) for j in range(3)] + [park]

                def outproj3_setup_tail(mt, nh, st):
                    def mm3():
                        st["ps"] = pp.tile([128, 512], F32, name="p3t",
                                           tag="pj", bufs=2)
                        nc.tensor.matmul(
                            st["ps"],
                            lhsT=ctxT_bf[3][mt // 4][
                                :, (mt % 4) * 128:(mt % 4 + 1) * 128],
                            rhs=wo_all[:, 3 * C + nh * 512:
                                       3 * C + (nh + 1) * 512],
                            start=True, stop=True)

                    def adddma():
                        ot = obuf.tile([128, 512], BF16, name="ot", tag="ot")
                        nc.vector.tensor_add(ot, st["ps"], st["po"])
                        eng = nc.sync if nh == 0 else nc.gpsimd
                        eng.dma_start(
                            out=out[mt * 128:(mt + 1) * 128,
                                    nh * 512:(nh + 1) * 512],
                            in_=ot)
                    st["mm3"] = mm3
                    st["adddma"] = adddma

                # ---- window stream ----
                # head: only pair 0's q/k + block-0 v run before attention
                # starts; everything else drains as deadline-ordered fillers.
                for u in proj_qk_uops(0, 0, wq_all, qT_bf):
                    u()
                for u in proj_qk_uops(0, 0, wk_all, kT_bf):
                    u()
                for m in range(4):
                    for u in proj_v_uops(m):
                        u()

                # filler queue per q-block phase, need-ordered:
                #   fill[0]: q/k for pairs 1-3 of block 0 (deadlines: their
                #            own jb0 windows), then pair 0 of block 1, then
                #            v(block 1) (needed by jb1's diagonal windows)
                #   fill[jb]: q/k pairs 1-3 of this block, pair 0 of next,
                #            v(next block)
                #   fill[3]: q/k pairs 1-3 of block 3, then all out-
                #            projections for blocks 0-2 (block 3 goes
                #            through the endgame path)
                def qk(jb, p):
                    return (proj_qk_uops(jb, p, wq_all, qT_bf)
                            + proj_qk_uops(jb, p, wk_all, kT_bf))

                op3_states = [{} for _ in range(8)]
                op3_groups = [(mt, nh) for mt in range(12, 16)
                              for nh in range(2)]

                # queue items are (uop, due, ready): `due` = (p, kt2) of
                # the first window whose scores/PV consume the uop's output
                # -- the loop FORCE-DRAINS everything due before emitting
                # that window, making correctness independent of pacing
                # (emission order IS execution order per engine; a consumer
                # emitted before its producer reads garbage).  `ready` =
                # earliest window the uop may be emitted (used by the op3
                # partials, which read pair 2's normalized ctx^T).
                fill = {jb: [] for jb in range(NJB)}
                for jb in range(NJB):
                    for p in range(1, npair):
                        fill[jb] += [(u, (p, 0), None) for u in qk(jb, p)]
                    if jb + 1 < NJB:
                        fill[jb] += [(u, None, None) for u in qk(jb + 1, 0)]
                        for m in range(4 * (jb + 1), 4 * (jb + 1) + 4):
                            fill[jb] += [(u, None, None)
                                         for u in proj_v_uops(m)]
                # 4 outproj groups of block 2 are held back to give the PE
                # work during the final pair's normalize chain; the op3
                # park-partials go last so they never emit before pair 2 of
                # block 3 is normalized (~3/4 through jb3).  op3 groups 0-3
                # take the direct-PSUM endgame path instead (their j=3 chunk
                # accumulates onto the partial inside PSUM, killing the DVE
                # add) using the score banks, which are free by then.
                held_back = []
                for b in range(NJB - 1):
                    for mt in range(4 * b, 4 * b + 4):
                        for nh in range(2):
                            g = outproj_uops(mt, nh)
                            if b == NJB - 2 and mt >= 4 * b + 2:
                                held_back.append(g)
                            else:
                                fill[NJB - 1] += [(u, None, None) for u in g]
                NDIRECT = 6
                for g in range(NDIRECT, 8):
                    fill[NJB - 1] += [(u, None, (3, 2)) for u in
                                      outproj3_part_uops(*op3_groups[g],
                                                         op3_states[g])]

                # pending PV work from the previous window: list of closures
                pend_pv = []
                pend_norm = []

                def emit_scores(jb, p, kt, c0, c1, nkt):
                    qoff = max(0, kt * 128 - jb * 512)
                    N = 512 - qoff
                    ps2 = pp.tile([128, 1024], F32, name="ps2", tag="pa",
                                  bufs=2)
                    kts = kT_bf[p][kt // 4][:, (kt % 4) * 128:
                                                (kt % 4 + 1) * 128]
                    qts = qT_bf[p][jb][:, qoff:qoff + N]
                    nc.tensor.matmul(
                        ps2[:, 0:N],
                        lhsT=kts[0:64, :], rhs=qts[0:64, :],
                        start=True, stop=True)
                    nc.tensor.matmul(
                        ps2[:, 512:512 + N],
                        lhsT=kts[64:128, :], rhs=qts[64:128, :],
                        start=True, stop=True, tile_position=(64, 0))
                    et = epool.tile([128, 1024], BF16, name="et", tag="et")
                    if qoff == 0:
                        nc.scalar.activation(et, ps2, AF.Exp, scale=0.125)
                    else:
                        e3 = et.rearrange("p (h x) -> p h x", x=512)
                        p3 = ps2.rearrange("p (h x) -> p h x", x=512)
                        nc.scalar.activation(
                            e3[:, :, 0:N], p3[:, :, 0:N],
                            AF.Exp, scale=0.125)
                    if kt * 128 >= jb * 512:
                        for hh in range(2):
                            nc.vector.tensor_mul(
                                et[:, hh * 512:hh * 512 + 128],
                                et[:, hh * 512:hh * 512 + 128],
                                tri)

                    def pv():
                        h0, h1 = 2 * p, 2 * p + 1
                        nc.tensor.matmul(
                            c0[:, qoff:512],
                            lhsT=v_bf[kt][:, h0 * VW:(h0 + 1) * VW],
                            rhs=et[:, 0:N],
                            start=(kt == 0), stop=(kt == nkt - 1))
                        nc.tensor.matmul(
                            c1[:, qoff:512],
                            lhsT=v_bf[kt][:, h1 * VW:(h1 + 1) * VW],
                            rhs=et[:, 512:512 + N],
                            start=(kt == 0), stop=(kt == nkt - 1))
                    return pv

                def emit_normalize_last(jb, p, c0, c1):
                    # endgame variant: nothing needs the ctx PSUM banks
                    # afterwards, so skip the evac-first ordering and put
                    # the reciprocals (read PSUM row 0 directly) at the
                    # head of the DVE queue — the partition broadcast then
                    # starts ~1.2us earlier and the whole chain shortens
                    cx0 = cfp.tile([128, 512], F32, name="cx0", tag="cx")
                    cx1 = cfp.tile([128, 512], F32, name="cx1", tag="cx")
                    rr2 = srp.tile([1, 1024], F32, name="rr2", tag="rr")
                    r64w = rbp.tile([128, 1024], F32, name="r64w", tag="rw")
                    nc.vector.reciprocal_approx_fast(
                        out=rr2[0:1, 0:512], in_=c0[0:1, :])
                    nc.vector.reciprocal_approx_fast(
                        out=rr2[0:1, 512:1024], in_=c1[0:1, :])
                    nc.vector.tensor_copy(cx0, c0)
                    nc.vector.tensor_copy(cx1, c1)
                    nc.gpsimd.partition_broadcast(r64w, rr2, channels=128)
                    nc.vector.tensor_mul(
                        ctxT_bf[p][jb][0:64, :], cx0[64:128, :],
                        r64w[64:128, 0:512])
                    nc.vector.tensor_mul(
                        ctxT_bf[p][jb][64:128, :], cx1[64:128, :],
                        r64w[64:128, 512:1024])

                def emit_normalize(jb, p, c0, c1):
                    # full-tile evacuations first: a [128,512] DVE copy costs
                    # the same as a [64,512] one (partition-parallel), and it
                    # frees each ctx PSUM bank in one op (~0.7us) instead of
                    # holding it through the reciprocal chain, so the next
                    # pair's first PV never waits.  The reciprocal then reads
                    # the denominator row from SBUF.
                    cx0 = cfp.tile([128, 512], F32, name="cx0", tag="cx")
                    cx1 = cfp.tile([128, 512], F32, name="cx1", tag="cx")
                    rr2 = srp.tile([1, 1024], F32, name="rr2", tag="rr")
                    r64w = rbp.tile([128, 1024], F32, name="r64w", tag="rw")
                    nc.vector.tensor_copy(cx0, c0)
                    nc.vector.tensor_copy(cx1, c1)
                    nc.vector.reciprocal_approx_fast(
                        out=rr2[0:1, 0:512], in_=cx0[0:1, :])
                    nc.vector.reciprocal_approx_fast(
                        out=rr2[0:1, 512:1024], in_=cx1[0:1, :])
                    nc.gpsimd.partition_broadcast(r64w, rr2, channels=128)
                    nc.vector.tensor_mul(
                        ctxT_bf[p][jb][0:64, :], cx0[64:128, :],
                        r64w[64:128, 0:512])
                    nc.vector.tensor_mul(
                        ctxT_bf[p][jb][64:128, :], cx1[64:128, :],
                        r64w[64:128, 512:1024])

                # global loop over window PAIRS: both score-pairs of two
                # consecutive key tiles are emitted back-to-back, then both
                # delayed PV batches.  Each transition between the row-tiled
                # score matmuls and full-array matmuls exposes one ~107ns
                # LDWEIGHTS (row-group conflict blocks the pull-ahead), so
                # halving the number of transitions saves ~110ns per window.
                for jb in range(NJB):
                    nkt = 4 * (jb + 1)
                    nwp = npair * nkt // 2
                    fillers = fill[jb]
                    it = 0
                    for p in range(npair):
                        c0 = pp.tile([128, 512], F32, name="c0", tag="pc",
                                     bufs=2)
                        c1 = pp.tile([128, 512], F32, name="c1", tag="pc",
                                     bufs=2)
                        for kt2 in range(0, nkt, 2):
                            # force-drain every filler due at this window
                            while (fillers and fillers[0][1] is not None
                                   and fillers[0][1] <= (p, kt2)):
                                fillers.pop(0)[0]()
                            pv_a = emit_scores(jb, p, kt2, c0, c1, nkt)
                            pv_b = emit_scores(jb, p, kt2 + 1, c0, c1, nkt)
                            # delayed PVs from the previous window pair
                            # (their exps finished a window ago -> no stall)
                            for f in pend_pv:
                                f()
                            pend_pv.clear()
                            for args in pend_norm:
                                emit_normalize(*args)
                            pend_norm.clear()
                            pend_pv += [pv_a, pv_b]
                            if kt2 + 1 == nkt - 1:
                                pend_norm.append((jb, p, c0, c1))
                            # evenly paced fillers
                            it += 1
                            if fillers:
                                nf = -(-len(fillers) // max(1, nwp - it + 1))
                                for _ in range(min(nf, 8, len(fillers))):
                                    if (fillers[0][2] is not None
                                            and fillers[0][2] > (p, kt2)):
                                        break
                                    fillers.pop(0)[0]()

                    # drain any leftover fillers before the next block
                    for u, _, _ in fillers:
                        u()
                    fill[jb] = []

                # endgame: flush the last PV batch, then the held-back
                # outproj matmuls (PE work that covers the final normalize
                # chain; their DVE copies are deferred so they don't delay
                # it), the fast last-pair normalize, the deferred copies,
                # and finally the j=3 tails + adds + stores
                for f in pend_pv:
                    f()
                pend_pv.clear()
                for g in range(NDIRECT, 8):
                    outproj3_setup_tail(*op3_groups[g], op3_states[g])
                # held-back groups: copies pipeline one group behind the
                # matmuls (on ScalarE) so their PSUM slots recycle without
                # gating the later direct-path work
                for i, grp in enumerate(held_back):
                    for u in grp[:-1]:
                        u()
                    if i >= 1:
                        held_back[i - 1][-1]("scalar")
                held_back[-1][-1]("scalar")

                def wo_sl(j, nh):
                    return wo_all[:, j * C + nh * 512:j * C + (nh + 1) * 512]

                def cx_sl(j, mt):
                    return ctxT_bf[j][mt // 4][:, (mt % 4) * 128:
                                               (mt % 4 + 1) * 128]

                # direct-path partials (j=0..2) into the freed score banks;
                # together with the held-back matmuls they keep the PE busy
                # through the final normalize chain
                direct_ps = []
                pa_t = None
                for g in range(NDIRECT):
                    mt, nh = op3_groups[g]
                    if g < 4:
                        if g % 2 == 0:
                            pa_t = pp.tile([128, 1024], F32, name="p3d",
                                           tag="pa", bufs=2)
                        ps = pa_t[:, (g % 2) * 512:(g % 2 + 1) * 512]
                    else:
                        ps = pp.tile([128, 512], F32, name="p3c", tag="pc",
                                     bufs=2)
                    direct_ps.append(ps)
                    for j in range(3):
                        nc.tensor.matmul(ps, lhsT=cx_sl(j, mt),
                                         rhs=wo_sl(j, nh), start=(j == 0),
                                         stop=(j == 2), skip_group_check=True)
                # pend_norm holds the last pair; use the fast variant
                emit_normalize_last(*pend_norm.pop())
                assert not pend_norm
                # direct tails first: accumulate j=3 onto the partial inside
                # PSUM, then a single copy + store.  Copies alternate
                # ScalarE/VectorE and store issues alternate the Sync/GpSimd
                # queues so neither serial path gates the drain.
                for g in range(NDIRECT):
                    mt, nh = op3_groups[g]
                    nc.tensor.matmul(direct_ps[g], lhsT=cx_sl(3, mt),
                                     rhs=wo_sl(3, nh), start=False,
                                     stop=True, skip_group_check=True)
                for g in range(NDIRECT, 8):
                    op3_states[g]["mm3"]()
                for g in range(NDIRECT):
                    mt, nh = op3_groups[g]
                    ot = obuf.tile([128, 512], BF16, name="ot", tag="ot")
                    if g % 2 == 0:
                        nc.scalar.activation(ot, direct_ps[g], AF.Copy,
                                             scale=1.0)
                    else:
                        nc.vector.tensor_copy(ot, direct_ps[g])
                    eng = nc.gpsimd if g % 2 == 0 else nc.sync
                    eng.dma_start(
                        out=out[mt * 128:(mt + 1) * 128,
                                nh * 512:(nh + 1) * 512],
                        in_=ot)
                for g in range(NDIRECT, 8):
                    op3_states[g]["adddma"]()

    nc.compile()
    return nc


_NC_CACHE = {}


def _get_nc(S=2048, npair=4):
    key = (S, npair)
    if key not in _NC_CACHE:
        _NC_CACHE[key] = build(S, npair)
    return _NC_CACHE[key]


def make_in_maps(x, Wq, Wk, Wv, Wo):
    """Host-side sharding: batch x head-group slices, x transposed to
    dims-major layout, rounded to bf16 (the dtype the device matmuls use)."""
    import ml_dtypes

    bf = ml_dtypes.bfloat16
    in_maps = []
    for c in range(8):
        b, g = divmod(c, 2)
        sl = slice(g * 512, (g + 1) * 512)
        in_maps.append({
            "xT": np.ascontiguousarray(x[b].T).astype(bf),
            "wq": np.ascontiguousarray(Wq[:, sl]).astype(bf),
            "wk": np.ascontiguousarray(Wk[:, sl]).astype(bf),
            "wv": np.ascontiguousarray(Wv[:, sl]).astype(bf),
            "wo": np.ascontiguousarray(Wo[sl, :]).astype(bf),
        })
    return in_maps


def run_cores(x, Wq, Wk, Wv, Wo, trace=False, trace_kwargs=None):
    nc = _get_nc(2048, 4)
    in_maps = make_in_maps(x, Wq, Wk, Wv, Wo)
    return run_bass_kernel_spmd(
        nc, in_maps, core_ids=list(range(8)), trace=trace,
        trace_kwargs=trace_kwargs or {})


def kernel(x, Wq, Wk, Wv, Wo, bo):
    x = np.asarray(x, dtype=np.float32)
    Wq = np.asarray(Wq, dtype=np.float32)
    Wk = np.asarray(Wk, dtype=np.float32)
    Wv = np.asarray(Wv, dtype=np.float32)
    Wo = np.asarray(Wo, dtype=np.float32)
    bo = np.asarray(bo, dtype=np.float32)

    res = run_cores(x, Wq, Wk, Wv, Wo).results
    out = np.empty((4, 2048, 1024), dtype=np.float32)
    for b in range(4):
        out[b] = (res[2 * b]["out"].astype(np.float32)
                  + res[2 * b + 1]["out"].astype(np.float32) + bo[None, :])
    return out
